# revision 1
# baseline (speedup 1.0000x reference)
"""Trainium2 Bass kernel for nn_CanonicalMicrocircuit (gnn_message_passing).

Math note: the reference module starts from all-zero recurrent state and only
returns `all_out * (1 - g)`, so every einsum against the zero state vanishes,
the inhibitory population and the inter-column lateral tensor are dead code,
and only layer 0 of the excitatory update survives:

    x0_c  = relu((1-exp(-1/tau_c)) * (blat_e[c,0] + bfb_e[c,0]) - thr_c)
    x0_c /= (||x0_c|| + 1e-8)
    out_c = relu(Wexc[c,0] @ x0_c + bexc[c,0])            # [H] per column
    h     = sum_c Wg1[:, cH:(c+1)H] @ out_c + bg1         # [H]
    r     = relu(h)
    g_c   = sigmoid(Wg2[cH:(c+1)H, :] @ r + bg2[cH:(c+1)H])
    final_c = out_c * (1 - g_c)                           # concat -> [C*H]

Sharding: one column per NeuronCore (C == 8 == n_cores).  Each core holds its
column's Wexc slice plus the matching column-block of Wg1 and row-block of
Wg2.  The only communication is one 4 KB AllGather of the per-core Wg1
partial products, summed locally on every core.

Engine plan (from profiling): the runtime inserts a collective-init barrier
on the CC stream at kernel entry (~46 us here) that also gates the Tensor
queue, and the ncfw AllGather costs ~39 us after trigger.  So stages A and B
run on DVE+GpSimd (scalar_tensor_tensor with accum_out = per-row dot
products against partition-broadcast vectors), pipelined behind the weight
DMAs and finishing before the barrier clears; the AllGather triggers as
early as its input exists; stage C (post-AllGather) is split between the PE
(rows 0-511, host-pre-transposed shard) and DVE (rows 512-1023, natural
shard) to shorten the tail.
"""

import numpy as np

import concourse.bass as bass
import concourse.bacc as bacc
import concourse.mybir as mybir
import concourse.tile as tile
from concourse.bass_utils import run_bass_kernel_spmd

C = 8
F = 512
L = 4
H = 1024
HI = 256
NCORES = 8
P = 128
KT = H // P  # 8 row/k tiles per 1024 dim
FP = mybir.dt.float32
TOP = 384  # stage-C rows on the PE
BOT = H - TOP
KB = BOT // P  # 5 DVE row-tiles in stage C

_CACHE = {}


def _build_nc():
    nc = bacc.Bacc(
        "TRN2",
        target_bir_lowering=False,
        debug=False,
        enable_asserts=False,
        num_devices=NCORES,
    )

    w1 = nc.dram_tensor("w1", [H, H], FP, kind="ExternalInput")  # Wexc[c,0] natural
    w2 = nc.dram_tensor("w2", [H, H], FP, kind="ExternalInput")  # Wg1[:,blk] natural
    w3t = nc.dram_tensor("w3t", [H, TOP], FP, kind="ExternalInput")  # top.T
    w3n = nc.dram_tensor("w3n", [BOT, H], FP, kind="ExternalInput")  # bottom nat
    vecs = nc.dram_tensor("vecs", [6, H], FP, kind="ExternalInput")
    eye = nc.dram_tensor("eye", [P, P], FP, kind="ExternalInput")
    # rows (rho = p-major storage permutation, see make_in_maps):
    # 0=blat, 1=bfb, 2=bexc[rho], 3=bg1[rho], 4=[bg2p[:512], bg2p-bot-col], 5=[tau, thr]
    fin = nc.dram_tensor("final", [1, H], FP, kind="ExternalOutput")

    AF = mybir.ActivationFunctionType
    ALU = mybir.AluOpType

    with tile.TileContext(nc) as tc:
        with (
            tc.tile_pool(name="sb", bufs=1) as sb,
            tc.tile_pool(name="jk", bufs=2) as jk,
            tc.tile_pool(name="ps_row", bufs=3, space="PSUM") as ps_row,
            tc.tile_pool(name="ps_tp", bufs=1, space="PSUM") as ps_tp,
            tc.tile_pool(name="dram", bufs=1, space="DRAM") as dram,
        ):
            # ---- weight loads: SP hwdge ring, FIFO in program order ----
            # W1 and W2 as 2x 2MB chunks (4 row-tiles each) for pipelining.
            def load_nat_pairs(name, dram_t):
                tiles = []
                for a in range(KT // 4):
                    t = sb.tile([P, 4, H], FP, tag=f"{name}{a}")
                    src = dram_t.ap()[4 * a * P : 4 * (a + 1) * P, :].rearrange(
                        "(t p) i -> p t i", p=P
                    )
                    nc.sync.dma_start(t[:], src)
                    tiles.append(t)
                return tiles  # tiles[a][:, b, :] is row-tile 4a+b

            w1_t = load_nat_pairs("w1", w1)
            w2_t = load_nat_pairs("w2", w2)
            w3t_t = sb.tile([P, KT, TOP], FP, tag="w3t")
            nc.sync.dma_start(w3t_t[:], w3t.ap().rearrange("(k p) i -> p k i", p=P))
            w3n_t = sb.tile([P, KB, H], FP, tag="w3n")
            nc.sync.dma_start(w3n_t[:], w3n.ap().rearrange("(t p) i -> p t i", p=P))

            # ---- small loads on the ACT hwdge ring ----
            vt = sb.tile([1, 6 * H], FP, tag="vecs")
            nc.scalar.dma_start(
                vt[:], vecs.ap().rearrange("a b -> (a b)").rearrange("(x n) -> x n", x=1)
            )
            bexc_col = sb.tile([P, KT], FP, tag="bexc_col")
            nc.scalar.dma_start(
                bexc_col[:], vecs.ap()[2].rearrange("(p t) -> p t", p=P)
            )
            bg2_bot = sb.tile([P, KB], FP, tag="bg2_bot")
            nc.scalar.dma_start(
                bg2_bot[:], vecs.ap()[4][TOP:H].rearrange("(p t) -> p t", p=P)
            )

            # ---- constants for the PE (post-collective stages) ----
            eye_t = sb.tile([P, P], FP, tag="eye")
            nc.scalar.dma_start(eye_t[:], eye.ap())
            ones_8 = sb.tile([KT, 1], FP, tag="ones_8")
            one_11 = sb.tile([1, 1], FP, tag="one_11")
            nc.vector.memset(ones_8[:], 1.0)
            nc.vector.memset(one_11[:], 1.0)

            # ---- x0 in row form on partition 0 ----
            rt = sb.tile([1, 1], FP, tag="rt")
            nc.vector.reciprocal(rt[:], vt[0:1, 5 * H : 5 * H + 1])
            ea = sb.tile([1, 1], FP, tag="ea")
            nc.scalar.activation(ea[:], rt[:], AF.Exp, scale=-1.0)  # exp(-1/tau)
            oma = sb.tile([1, 1], FP, tag="oma")
            nc.scalar.activation(oma[:], ea[:], AF.Copy, scale=-1.0, bias=1.0)
            nthr = sb.tile([1, 1], FP, tag="nthr")
            nc.scalar.activation(nthr[:], vt[0:1, 5 * H + 1 : 5 * H + 2], AF.Copy, scale=-1.0)

            xr = sb.tile([1, H], FP, tag="xr")
            nc.vector.tensor_add(xr[:], vt[0:1, 0:H], vt[0:1, H : 2 * H])
            nc.vector.tensor_scalar(
                xr[:], xr[:], oma[:], nthr[:], op0=ALU.mult, op1=ALU.add
            )
            nc.vector.tensor_scalar_max(xr[:], xr[:], 0.0)
            ssq = sb.tile([1, 1], FP, tag="ssq")
            sqj = jk.tile([1, H], FP, tag="sqj")
            nc.vector.scalar_tensor_tensor(
                sqj[:], xr[:], 1.0, xr[:], op0=ALU.mult, op1=ALU.mult,
                accum_out=ssq[:],
            )
            nrm = sb.tile([1, 1], FP, tag="nrm")
            nc.scalar.activation(nrm[:], ssq[:], AF.Sqrt)
            nc.scalar.activation(nrm[:], nrm[:], AF.Copy, bias=1e-8)
            inv = sb.tile([1, 1], FP, tag="inv")
            nc.vector.reciprocal(inv[:], nrm[:])
            nc.vector.tensor_scalar_mul(xr[:], xr[:], inv[:])

            xb = sb.tile([P, H], FP, tag="xb")
            nc.gpsimd.partition_broadcast(xb[:], xr[0:1, :])

            # ---- fused row-dot matvec: acc[p, t] = sum_j W[t*128+p, j]*v[j]
            def matvec_nat(tiles, vb, acc):
                for t in range(KT):
                    w_ap = tiles[t // 4][:, t % 4, :]
                    junk = jk.tile([P, H], FP, tag="jv")
                    nc.vector.scalar_tensor_tensor(
                        junk[:], w_ap, 1.0, vb[:], op0=ALU.mult, op1=ALU.mult,
                        accum_out=acc[:, t : t + 1],
                    )

            # Stage A: out_c = relu(W1 @ x0 + bexc)
            outa = sb.tile([P, KT], FP, tag="outa")
            matvec_nat(w1_t, xb, outa)
            nc.vector.tensor_add(outa[:], outa[:], bexc_col[:])
            nc.vector.tensor_scalar_max(outa[:], outa[:], 0.0)
            outa_row = sb.tile([1, H], FP, tag="outa_row")
            nc.scalar.dma_start(outa_row[:], outa[:])
            xb2 = sb.tile([P, H], FP, tag="xb2")
            nc.gpsimd.partition_broadcast(xb2[:], outa_row[0:1, :])

            # Stage B: hp = W2 @ out_c + bg1/8 (so the gathered sum includes bg1)
            bg1_col = sb.tile([P, KT], FP, tag="bg1_col")
            nc.scalar.dma_start(
                bg1_col[:], vecs.ap()[3].rearrange("(p t) -> p t", p=P)
            )
            hp = sb.tile([P, KT], FP, tag="hp")
            matvec_nat(w2_t, xb2, hp)
            nc.vector.scalar_tensor_tensor(
                hp[:], bg1_col[:], 0.125, hp[:], op0=ALU.mult, op1=ALU.add
            )

            # AllGather the 4KB partials, triggered as soon as hp lands
            cc_in = dram.tile([1, H], FP, tag="cc_in")
            cc_out = dram.tile([NCORES, H], FP, tag="cc_out")
            nc.scalar.dma_start(cc_in[:], hp[:])
            nc.gpsimd.collective_compute(
                "AllGather",
                ALU.bypass,
                replica_groups=[list(range(NCORES))],
                ins=[cc_in[:]],
                outs=[cc_out[:]],
            )
            # r = relu(sum_c partials) straight into PE-ready column form:
            # per k-tile, lhsT = agt slice [8, 128] summed by a ones rhs.
            agt_a = sb.tile([NCORES, H // 2], FP, tag="agt_a")
            nc.scalar.dma_start(agt_a[:], cc_out[:, 0 : H // 2])
            agt_b = sb.tile([NCORES, H // 2], FP, tag="agt_b")
            nc.sync.dma_start(agt_b[:], cc_out[:, H // 2 : H])
            psRc = ps_tp.tile([P, KT], FP, tag="tp")
            for t in range(KT):
                half = agt_a if t < 4 else agt_b
                nc.tensor.matmul(
                    psRc[:, t : t + 1],
                    half[:, (t % 4) * P : (t % 4 + 1) * P],
                    ones_8[:],
                    start=True,
                    stop=True,
                )
            r_col = sb.tile([P, KT], FP, tag="r_col")
            nc.scalar.activation(r_col[:], psRc[:], AF.Relu)

            # Stage C rows 0-511 on the PE: s = sigmoid(-(W3 @ r + bg2))

            s_row = sb.tile([1, H], FP, tag="s_row")
            ps = ps_row.tile([1, TOP], FP, tag="row")
            for k in range(KT):
                nc.tensor.matmul(
                    ps[:],
                    r_col[:, k : k + 1],
                    w3t_t[:, k, :],
                    start=(k == 0),
                    stop=False,
                )
            nc.tensor.matmul(
                ps[:], one_11[:], vt[0:1, 4 * H : 4 * H + TOP], start=False, stop=True
            )
            nc.scalar.activation(s_row[0:1, 0:TOP], ps[:], AF.Sigmoid, scale=-1.0)

            # Stage C rows 512-1023 on DVE/GpSimd
            rrow2 = sb.tile([1, H], FP, tag="rrow2")
            nc.scalar.dma_start(rrow2[:], r_col[:])
            xb3 = sb.tile([P, H], FP, tag="xb3")
            nc.gpsimd.partition_broadcast(xb3[:], rrow2[0:1, :])
            zb = sb.tile([P, KB], FP, tag="zb")
            for t in range(KB):
                junk = jk.tile([P, H], FP, tag="jv")
                nc.vector.scalar_tensor_tensor(
                    junk[:], w3n_t[:, t, :], 1.0, xb3[:], op0=ALU.mult, op1=ALU.mult,
                    accum_out=zb[:, t : t + 1],
                )
            nc.vector.tensor_add(zb[:], zb[:], bg2_bot[:])
            # row-ize zb on the PE (identity transpose), sigmoid from PSUM
            psZ = ps_row.tile([1, BOT], FP, tag="row")
            for t in range(KB):
                nc.tensor.matmul(
                    psZ[0:1, t * P : (t + 1) * P],
                    zb[:, t : t + 1],
                    eye_t[:],
                    start=True,
                    stop=True,
                )
            nc.scalar.activation(s_row[0:1, TOP:H], psZ[:], AF.Sigmoid, scale=-1.0)

            # final = out_c * s  (rho-ordered row; host un-permutes)
            fin_a = sb.tile([1, TOP], FP, tag="fin_a")
            nc.vector.tensor_mul(fin_a[:], outa_row[0:1, 0:TOP], s_row[0:1, 0:TOP])
            nc.sync.dma_start(fin.ap()[0:1, 0:TOP], fin_a[:])
            fin_b = sb.tile([1, BOT], FP, tag="fin_b")
            nc.vector.tensor_mul(fin_b[:], outa_row[0:1, TOP:H], s_row[0:1, TOP:H])
            nc.sync.dma_start(fin.ap()[0:1, TOP:H], fin_b[:])

    nc.compile()
    return nc


def get_nc():
    if "nc" not in _CACHE:
        _CACHE["nc"] = _build_nc()
    return _CACHE["nc"]


def make_in_maps(inputs):
    """Slice the full inputs into 8 per-core input dicts (layout prep only).

    RHO is the p-major storage permutation: the device keeps the exchanged
    1024-vectors in storage order s with natural index rho[s] = (s%8)*128 +
    s//8, which makes every on-device transpose DMA contiguous.  The
    contractions are order-invariant, so we permute the matching weight
    columns / bias entries here and un-permute the final output on the host.
    """
    Wexc = np.asarray(inputs["Wexc"], dtype=np.float32)
    Wg1 = np.asarray(inputs["Wg1"], dtype=np.float32)
    Wg2 = np.asarray(inputs["Wg2"], dtype=np.float32)
    blat = np.asarray(inputs["blat_e"], dtype=np.float32)
    bfb = np.asarray(inputs["bfb_e"], dtype=np.float32)
    bexc = np.asarray(inputs["bexc"], dtype=np.float32)
    bg1 = np.asarray(inputs["bg1"], dtype=np.float32)
    bg2 = np.asarray(inputs["bg2"], dtype=np.float32)
    tau = np.asarray(inputs["tau_exc"], dtype=np.float32)
    thr = np.asarray(inputs["threshold"], dtype=np.float32)

    s_idx = np.arange(H)
    rho = (s_idx % KT) * P + s_idx // KT  # storage -> natural
    eye = np.eye(P, dtype=np.float32)

    in_maps = []
    for c in range(NCORES):
        sl = slice(c * H, (c + 1) * H)
        srow = np.zeros((H,), np.float32)
        srow[0], srow[1] = tau[c], thr[c]
        bg2p = bg2[sl][rho]
        # bottom col-form bias: row4[TOP + p*KB + t] = bg2p[TOP + t*128 + p]
        bg2_bot = bg2p[TOP:].reshape(KB, P).T.reshape(-1)
        row4 = np.concatenate([bg2p[:TOP], bg2_bot])
        vecs = np.stack([blat[c, 0], bfb[c, 0], bexc[c, 0][rho], bg1[rho], row4, srow])
        w3pp = Wg2[sl][np.ix_(rho, rho)]
        in_maps.append(
            {
                "w1": np.ascontiguousarray(Wexc[c, 0]),
                "w2": np.ascontiguousarray(Wg1[:, sl][:, rho]),
                "w3t": np.ascontiguousarray(w3pp[0:TOP, :].T),
                "w3n": np.ascontiguousarray(w3pp[TOP:, :][:, rho]),
                "vecs": np.ascontiguousarray(vecs),
                "eye": eye,
            }
        )
    return in_maps


def kernel(**inputs):
    nc = get_nc()
    in_maps = make_in_maps(inputs)
    res = run_bass_kernel_spmd(nc, in_maps, core_ids=list(range(NCORES)))
    _CACHE["last_result"] = res
    chunks = []
    for c in range(NCORES):
        st = res.results[c]["final"].reshape(P, KT)  # storage s = p*KT + t
        chunks.append(np.ascontiguousarray(st.T).reshape(-1))  # natural t*P+p
    return np.concatenate(chunks).astype(np.float32)



# revision 18
# speedup vs baseline: 1.3432x; 1.3432x over previous
"""Trainium2 Bass kernel for nn_CanonicalMicrocircuit (gnn_message_passing).

Math note: the reference module starts from all-zero recurrent state and only
returns `all_out * (1 - g)`, so every einsum against the zero state vanishes,
the inhibitory population and the inter-column lateral tensor are dead code,
and only layer 0 of the excitatory update survives:

    x0_c  = relu((1-exp(-1/tau_c)) * (blat_e[c,0] + bfb_e[c,0]) - thr_c)
    x0_c /= (||x0_c|| + 1e-8)
    out_c = relu(Wexc[c,0] @ x0_c + bexc[c,0])            # [H] per column
    h     = sum_c Wg1[:, cH:(c+1)H] @ out_c + bg1         # [H]
    r     = relu(h)
    g_c   = sigmoid(Wg2[cH:(c+1)H, :] @ r + bg2[cH:(c+1)H])
    final_c = out_c * (1 - g_c)                           # concat -> [C*H]

Fast path (v3): for the reference input distribution the layer-0 drive
(1-a)*(blat+bfb) tops out at ~0.26 while the threshold is 0.5, so x0 is
EXACTLY zero and out_c = relu(bexc[c,0]) on every column — verified on the
host for the actual inputs before this path is taken (guard below; the
collective-based general kernel is kept as fallback).  With out_all local to
every core there is no cross-core dataflow at all: each core redundantly
computes r = relu(bg1 + Wg1 @ out_all) (full Wg1 in fp8e4m3, 8MB, streamed
through the PE as 128 fp8 matmuls) and then its own output block
g_c/final_c (Wg2 block in bf16, split PE/DVE).  No collectives, no remote
DMA, no inter-core waits: immune to the multi-ms core-launch skew this
runtime exhibits for collective-free NEFFs, and each core's profile is just
its own ~35us of local work.  fp8 Wg1 + bf16 Wg2 gives rel err ~1.0e-2
against the fp32 reference (budget 2e-2); everything else stays fp32.

Fallback path: the previous collective-based kernel (ncfw AllGather of the
hp partials), used only if the host guard ever finds x0 != 0.
"""

import numpy as np
import ml_dtypes

import concourse.bass as bass
import concourse.bacc as bacc
import concourse.mybir as mybir
import concourse.tile as tile
from concourse.bass_utils import run_bass_kernel_spmd

C = 8
F = 512
L = 4
H = 1024
NCORES = 8
P = 128
KT = H // P       # 8 tiles per 1024 dim
KG = (C * H) // P  # 64 k-tiles over the 8192 contraction dim
NCH = 8            # wg1 DMA chunks (8 k-tiles each)
TOP = 512          # stage-C rows on the PE
BOT = H - TOP
KB = BOT // P
FP = mybir.dt.float32
BF = mybir.dt.bfloat16
F8 = mybir.dt.float8e4
NPBF = np.dtype(ml_dtypes.bfloat16)
NPF8 = np.dtype(ml_dtypes.float8_e4m3)

_CACHE = {}


def _build_nc_fast():
    nc = bacc.Bacc(
        "TRN2",
        target_bir_lowering=False,
        debug=False,
        enable_asserts=False,
        num_devices=NCORES,
    )

    wg1t = nc.dram_tensor("wg1t", [C * H, H], F8, kind="ExternalInput")  # Wg1.T
    w3t = nc.dram_tensor("w3t", [H, TOP], BF, kind="ExternalInput")  # Wg2[blk][:TOP].T
    w3n = nc.dram_tensor("w3n", [BOT, H], BF, kind="ExternalInput")  # Wg2[blk][TOP:]
    bxc = nc.dram_tensor("bxc", [P, KG], FP, kind="ExternalInput")  # vec(bexc[:,0]) col
    vecs = nc.dram_tensor("vecs", [2, H], FP, kind="ExternalInput")
    # rows: 0 = bexc[c,0] (this block's out pre-relu), 1 = bg1
    colsb = nc.dram_tensor("colsb", [P, KB], FP, kind="ExternalInput")  # bg2 bot col
    b16 = nc.dram_tensor("b16", [1, TOP], BF, kind="ExternalInput")  # bg2[:TOP] bf16
    eye = nc.dram_tensor("eye", [P, P], FP, kind="ExternalInput")
    fin = nc.dram_tensor("final", [1, H], FP, kind="ExternalOutput")

    AF = mybir.ActivationFunctionType
    ALU = mybir.AluOpType

    with tile.TileContext(nc) as tc:
        with (
            tc.tile_pool(name="sb", bufs=1) as sb,
            tc.tile_pool(name="jk", bufs=2) as jk,
            tc.tile_pool(name="ps_bc", bufs=1, space="PSUM") as ps_bc,
            tc.tile_pool(name="ps_row", bufs=1, space="PSUM") as ps_row,
            tc.tile_pool(name="ps_r", bufs=1, space="PSUM") as ps_r,
            tc.tile_pool(name="ps_top", bufs=1, space="PSUM") as ps_top,
        ):
            # ---- small loads first on the ACT ring ----
            vt = sb.tile([1, 2 * H], FP, tag="vecs")
            nc.scalar.dma_start(
                vt[:], vecs.ap().rearrange("a b -> (a b)").rearrange("(x n) -> x n", x=1)
            )
            bxc_t = sb.tile([P, KG], FP, tag="bxc")
            nc.scalar.dma_start(bxc_t[:], bxc.ap())
            cols_t = sb.tile([P, KB], FP, tag="colsb")
            nc.scalar.dma_start(cols_t[:], colsb.ap())
            eye_t = sb.tile([P, P], FP, tag="eye")
            nc.scalar.dma_start(eye_t[:], eye.ap())
            b16_t = sb.tile([1, TOP], BF, tag="b16")
            nc.scalar.dma_start(b16_t[:], b16.ap())

            # act-table prewarm (Sigmoid/Relu/Copy) so no load lands mid-tail
            warm = sb.tile([1, 1], FP, tag="warm")
            nc.vector.memset(warm[:], 0.3)
            wj = sb.tile([1, 3], FP, tag="wj")
            for i, fn in enumerate((AF.Sigmoid, AF.Relu, AF.Copy)):
                nc.scalar.activation(wj[0:1, i : i + 1], warm[:], fn)

            ones_rb = sb.tile([1, P], BF, tag="ones_rb")
            one_11b = sb.tile([1, 1], BF, tag="one_11b")
            nc.vector.memset(ones_rb[:], 1.0)
            nc.vector.memset(one_11b[:], 1.0)

            # ---- big loads: wg1 chunks alternate rings, wg2 parts last ----
            wch = []
            for a in range(NCH):
                t = sb.tile([P, KG // NCH, H], F8, tag=f"wg1_{a}")
                src = wg1t.ap()[
                    a * (KG // NCH) * P : (a + 1) * (KG // NCH) * P, :
                ].rearrange("(k p) i -> p k i", p=P)
                eng = nc.sync if a % 2 == 0 else nc.scalar
                eng.dma_start(t[:], src)
                wch.append(t)
            w3t_t = sb.tile([P, KT, TOP], BF, tag="w3t")
            nc.sync.dma_start(w3t_t[:], w3t.ap().rearrange("(k p) i -> p k i", p=P))
            w3n_t = sb.tile([P, KB, H], BF, tag="w3n")
            nc.scalar.dma_start(w3n_t[:], w3n.ap().rearrange("(t p) i -> p t i", p=P))

            # ---- out_all = relu(bexc[:,0,:]) in fp8 col form; out_c row ----
            oa_f = sb.tile([P, KG], FP, tag="oa_f")
            nc.vector.tensor_scalar_max(oa_f[:], bxc_t[:], 0.0)
            out8 = sb.tile([P, KG], F8, tag="out8")
            nc.scalar.activation(out8[:], oa_f[:], AF.Copy)
            out_row = sb.tile([1, H], FP, tag="out_row")
            nc.vector.tensor_scalar_max(out_row[:], vt[0:1, 0:H], 0.0)

            # ---- r = relu(bg1 + Wg1 @ out_all): 128 fp8 matmuls on the PE ----
            psr0 = ps_r.tile([1, TOP], FP, tag="r0")
            psr1 = ps_r.tile([1, TOP], FP, tag="r1")
            psr = [psr0, psr1]
            for k in range(KG):
                a, b = k // (KG // NCH), k % (KG // NCH)
                for h in range(2):
                    nc.tensor.matmul(
                        psr[h][:],
                        out8[:, k : k + 1],
                        wch[a][:, b, h * TOP : (h + 1) * TOP],
                        start=(k == 0),
                        stop=(k == KG - 1),
                    )
            r_row = sb.tile([1, H], FP, tag="r_row")
            for h in range(2):
                sl = slice(h * TOP, (h + 1) * TOP)
                nc.vector.tensor_add(r_row[0:1, sl], psr[h][:], vt[0:1, H + h * TOP : H + (h + 1) * TOP])
            nc.vector.tensor_scalar_max(r_row[:], r_row[:], 0.0)
            rrowb = sb.tile([1, H], BF, tag="rrowb")
            nc.scalar.activation(rrowb[:], r_row[:], AF.Copy)

            # ---- r row -> col (PE), and broadcast for the DVE bottom half ----
            psc = ps_row.tile([P, KT], FP, tag="rcol")
            for t in range(KT):
                nc.tensor.matmul(
                    psc[:, t : t + 1],
                    rrowb[0:1, t * P : (t + 1) * P],
                    one_11b[:],
                    start=True,
                    stop=True,
                )
            r_colb = sb.tile([P, KT], BF, tag="r_colb")
            nc.scalar.activation(r_colb[:], psc[:], AF.Copy)

            psb = ps_bc.tile([P, H], FP, tag="bc")
            for h in range(2):
                nc.tensor.matmul(
                    psb[:, h * TOP : (h + 1) * TOP],
                    ones_rb[:],
                    rrowb[0:1, h * TOP : (h + 1) * TOP],
                    start=True,
                    stop=True,
                )
            xb3 = sb.tile([P, H], BF, tag="xb3")
            nc.scalar.activation(xb3[:], psb[:], AF.Copy)

            # ---- stage C top on PE: s = sigmoid(-(W3[:TOP] @ r + bg2[:TOP]))
            s_row = sb.tile([1, H], FP, tag="s_row")
            ps = ps_top.tile([1, TOP], FP, tag="top")
            for k in range(KT):
                nc.tensor.matmul(
                    ps[:], r_colb[:, k : k + 1], w3t_t[:, k, :],
                    start=(k == 0), stop=False,
                )
            nc.tensor.matmul(ps[:], one_11b[:], b16_t[:], start=False, stop=True)
            nc.scalar.activation(s_row[0:1, 0:TOP], ps[:], AF.Sigmoid, scale=-1.0)

            # ---- stage C bottom on DVE ----
            zb = sb.tile([P, KB], FP, tag="zb")
            for t in range(KB):
                junk = jk.tile([P, H], BF, tag="jv")
                nc.vector.scalar_tensor_tensor(
                    junk[:], w3n_t[:, t, :], 1.0, xb3[:], op0=ALU.mult, op1=ALU.mult,
                    accum_out=zb[:, t : t + 1],
                )
            nc.vector.tensor_add(zb[:], zb[:], cols_t[:])
            psZ = ps_row.tile([1, H], FP, tag="zrow")
            for t in range(KB):
                nc.tensor.matmul(
                    psZ[0:1, t * P : (t + 1) * P], zb[:, t : t + 1], eye_t[:],
                    start=True, stop=True,
                )
            nc.scalar.activation(
                s_row[0:1, TOP:H], psZ[0:1, 0:BOT], AF.Sigmoid, scale=-1.0
            )

            # final = out_c * s
            fin_a = sb.tile([1, TOP], FP, tag="fin_a")
            nc.vector.tensor_mul(fin_a[:], out_row[0:1, 0:TOP], s_row[0:1, 0:TOP])
            nc.sync.dma_start(fin.ap()[0:1, 0:TOP], fin_a[:])
            fin_b = sb.tile([1, BOT], FP, tag="fin_b")
            nc.vector.tensor_mul(fin_b[:], out_row[0:1, TOP:H], s_row[0:1, TOP:H])
            nc.sync.dma_start(fin.ap()[0:1, TOP:H], fin_b[:])

    nc.compile()
    return nc


def _make_in_maps_fast(inputs):
    bexc = np.asarray(inputs["bexc"], dtype=np.float32)
    bg1 = np.asarray(inputs["bg1"], dtype=np.float32)
    bg2 = np.asarray(inputs["bg2"], dtype=np.float32)
    Wg1 = np.asarray(inputs["Wg1"], dtype=np.float32)
    Wg2 = np.asarray(inputs["Wg2"], dtype=np.float32)

    wg1t = np.ascontiguousarray(Wg1.T).astype(NPF8)  # [C*H, H], shared
    bx_flat = bexc[:, 0, :].reshape(-1)
    bxc = np.ascontiguousarray(bx_flat.reshape(KG, P).T)  # [P, KG], shared
    eye = np.eye(P, dtype=np.float32)

    in_maps = []
    for c in range(NCORES):
        sl = slice(c * H, (c + 1) * H)
        w3 = Wg2[sl]
        vecs = np.stack([bexc[c, 0], bg1])
        in_maps.append(
            {
                "wg1t": wg1t,
                "w3t": np.ascontiguousarray(w3[0:TOP, :].T).astype(NPBF),
                "w3n": np.ascontiguousarray(w3[TOP:, :]).astype(NPBF),
                "bxc": bxc,
                "vecs": np.ascontiguousarray(vecs),
                "colsb": np.ascontiguousarray(bg2[sl][TOP:].reshape(KB, P).T),
                "b16": np.ascontiguousarray(bg2[sl][:TOP])[None, :].astype(NPBF),
                "eye": eye,
            }
        )
    return in_maps


# ---------------------------------------------------------------------------
# Fallback: collective-based general kernel (previous proven version), used
# only if the x0==0 guard fails.
# ---------------------------------------------------------------------------

HI = 256
RTOP = 384  # stage-C rows on the PE (fallback layout)
RBOT = H - RTOP
RKB = RBOT // P


def _build_nc_ref():
    nc = bacc.Bacc(
        "TRN2",
        target_bir_lowering=False,
        debug=False,
        enable_asserts=False,
        num_devices=NCORES,
    )

    w1 = nc.dram_tensor("w1", [H, H], FP, kind="ExternalInput")
    w2 = nc.dram_tensor("w2", [H, H], FP, kind="ExternalInput")
    w3t = nc.dram_tensor("w3t", [H, RTOP], FP, kind="ExternalInput")
    w3n = nc.dram_tensor("w3n", [RBOT, H], FP, kind="ExternalInput")
    vecs = nc.dram_tensor("vecs", [6, H], FP, kind="ExternalInput")
    eye = nc.dram_tensor("eye", [P, P], FP, kind="ExternalInput")
    fin = nc.dram_tensor("final", [1, H], FP, kind="ExternalOutput")

    AF = mybir.ActivationFunctionType
    ALU = mybir.AluOpType

    with tile.TileContext(nc) as tc:
        with (
            tc.tile_pool(name="sb", bufs=1) as sb,
            tc.tile_pool(name="jk", bufs=2) as jk,
            tc.tile_pool(name="ps_row", bufs=3, space="PSUM") as ps_row,
            tc.tile_pool(name="ps_tp", bufs=1, space="PSUM") as ps_tp,
            tc.tile_pool(name="dram", bufs=1, space="DRAM") as dram,
        ):
            def load_nat_pairs(name, dram_t):
                tiles = []
                for a in range(KT // 4):
                    t = sb.tile([P, 4, H], FP, tag=f"{name}{a}")
                    src = dram_t.ap()[4 * a * P : 4 * (a + 1) * P, :].rearrange(
                        "(t p) i -> p t i", p=P
                    )
                    nc.sync.dma_start(t[:], src)
                    tiles.append(t)
                return tiles

            w1_t = load_nat_pairs("w1", w1)
            w2_t = load_nat_pairs("w2", w2)
            w3t_t = sb.tile([P, KT, RTOP], FP, tag="w3t")
            nc.sync.dma_start(w3t_t[:], w3t.ap().rearrange("(k p) i -> p k i", p=P))
            w3n_t = sb.tile([P, RKB, H], FP, tag="w3n")
            nc.sync.dma_start(w3n_t[:], w3n.ap().rearrange("(t p) i -> p t i", p=P))

            vt = sb.tile([1, 6 * H], FP, tag="vecs")
            nc.scalar.dma_start(
                vt[:], vecs.ap().rearrange("a b -> (a b)").rearrange("(x n) -> x n", x=1)
            )
            bexc_col = sb.tile([P, KT], FP, tag="bexc_col")
            nc.scalar.dma_start(
                bexc_col[:], vecs.ap()[2].rearrange("(p t) -> p t", p=P)
            )
            bg2_bot = sb.tile([P, RKB], FP, tag="bg2_bot")
            nc.scalar.dma_start(
                bg2_bot[:], vecs.ap()[4][RTOP:H].rearrange("(p t) -> p t", p=P)
            )

            eye_t = sb.tile([P, P], FP, tag="eye")
            nc.scalar.dma_start(eye_t[:], eye.ap())
            ones_8 = sb.tile([KT, 1], FP, tag="ones_8")
            one_11 = sb.tile([1, 1], FP, tag="one_11")
            nc.vector.memset(ones_8[:], 1.0)
            nc.vector.memset(one_11[:], 1.0)

            rt = sb.tile([1, 1], FP, tag="rt")
            nc.vector.reciprocal(rt[:], vt[0:1, 5 * H : 5 * H + 1])
            ea = sb.tile([1, 1], FP, tag="ea")
            nc.scalar.activation(ea[:], rt[:], AF.Exp, scale=-1.0)
            oma = sb.tile([1, 1], FP, tag="oma")
            nc.scalar.activation(oma[:], ea[:], AF.Copy, scale=-1.0, bias=1.0)
            nthr = sb.tile([1, 1], FP, tag="nthr")
            nc.scalar.activation(nthr[:], vt[0:1, 5 * H + 1 : 5 * H + 2], AF.Copy, scale=-1.0)

            xr = sb.tile([1, H], FP, tag="xr")
            nc.vector.tensor_add(xr[:], vt[0:1, 0:H], vt[0:1, H : 2 * H])
            nc.vector.tensor_scalar(
                xr[:], xr[:], oma[:], nthr[:], op0=ALU.mult, op1=ALU.add
            )
            nc.vector.tensor_scalar_max(xr[:], xr[:], 0.0)
            ssq = sb.tile([1, 1], FP, tag="ssq")
            sqj = jk.tile([1, H], FP, tag="sqj")
            nc.vector.scalar_tensor_tensor(
                sqj[:], xr[:], 1.0, xr[:], op0=ALU.mult, op1=ALU.mult,
                accum_out=ssq[:],
            )
            nrm = sb.tile([1, 1], FP, tag="nrm")
            nc.scalar.activation(nrm[:], ssq[:], AF.Sqrt)
            nc.scalar.activation(nrm[:], nrm[:], AF.Copy, bias=1e-8)
            inv = sb.tile([1, 1], FP, tag="inv")
            nc.vector.reciprocal(inv[:], nrm[:])
            nc.vector.tensor_scalar_mul(xr[:], xr[:], inv[:])

            xb = sb.tile([P, H], FP, tag="xb")
            nc.gpsimd.partition_broadcast(xb[:], xr[0:1, :])

            def matvec_nat(tiles, vb, acc):
                for t in range(KT):
                    w_ap = tiles[t // 4][:, t % 4, :]
                    junk = jk.tile([P, H], FP, tag="jv")
                    nc.vector.scalar_tensor_tensor(
                        junk[:], w_ap, 1.0, vb[:], op0=ALU.mult, op1=ALU.mult,
                        accum_out=acc[:, t : t + 1],
                    )

            outa = sb.tile([P, KT], FP, tag="outa")
            matvec_nat(w1_t, xb, outa)
            nc.vector.tensor_add(outa[:], outa[:], bexc_col[:])
            nc.vector.tensor_scalar_max(outa[:], outa[:], 0.0)
            outa_row = sb.tile([1, H], FP, tag="outa_row")
            nc.scalar.dma_start(outa_row[:], outa[:])
            xb2 = sb.tile([P, H], FP, tag="xb2")
            nc.gpsimd.partition_broadcast(xb2[:], outa_row[0:1, :])

            bg1_col = sb.tile([P, KT], FP, tag="bg1_col")
            nc.scalar.dma_start(
                bg1_col[:], vecs.ap()[3].rearrange("(p t) -> p t", p=P)
            )
            hp = sb.tile([P, KT], FP, tag="hp")
            matvec_nat(w2_t, xb2, hp)
            nc.vector.scalar_tensor_tensor(
                hp[:], bg1_col[:], 0.125, hp[:], op0=ALU.mult, op1=ALU.add
            )

            cc_in = dram.tile([1, H], FP, tag="cc_in")
            cc_out = dram.tile([NCORES, H], FP, tag="cc_out")
            nc.scalar.dma_start(cc_in[:], hp[:])
            nc.gpsimd.collective_compute(
                "AllGather",
                ALU.bypass,
                replica_groups=[list(range(NCORES))],
                ins=[cc_in[:]],
                outs=[cc_out[:]],
            )
            agt_a = sb.tile([NCORES, H // 2], FP, tag="agt_a")
            nc.scalar.dma_start(agt_a[:], cc_out[:, 0 : H // 2])
            agt_b = sb.tile([NCORES, H // 2], FP, tag="agt_b")
            nc.sync.dma_start(agt_b[:], cc_out[:, H // 2 : H])
            psRc = ps_tp.tile([P, KT], FP, tag="tp")
            for t in range(KT):
                half = agt_a if t < 4 else agt_b
                nc.tensor.matmul(
                    psRc[:, t : t + 1],
                    half[:, (t % 4) * P : (t % 4 + 1) * P],
                    ones_8[:],
                    start=True,
                    stop=True,
                )
            r_col = sb.tile([P, KT], FP, tag="r_col")
            nc.scalar.activation(r_col[:], psRc[:], AF.Relu)

            s_row = sb.tile([1, H], FP, tag="s_row")
            ps = ps_row.tile([1, RTOP], FP, tag="row")
            for k in range(KT):
                nc.tensor.matmul(
                    ps[:],
                    r_col[:, k : k + 1],
                    w3t_t[:, k, :],
                    start=(k == 0),
                    stop=False,
                )
            nc.tensor.matmul(
                ps[:], one_11[:], vt[0:1, 4 * H : 4 * H + RTOP], start=False, stop=True
            )
            nc.scalar.activation(s_row[0:1, 0:RTOP], ps[:], AF.Sigmoid, scale=-1.0)

            rrow2 = sb.tile([1, H], FP, tag="rrow2")
            nc.scalar.dma_start(rrow2[:], r_col[:])
            xb3 = sb.tile([P, H], FP, tag="xb3")
            nc.gpsimd.partition_broadcast(xb3[:], rrow2[0:1, :])
            zb = sb.tile([P, RKB], FP, tag="zb")
            for t in range(RKB):
                junk = jk.tile([P, H], FP, tag="jv")
                nc.vector.scalar_tensor_tensor(
                    junk[:], w3n_t[:, t, :], 1.0, xb3[:], op0=ALU.mult, op1=ALU.mult,
                    accum_out=zb[:, t : t + 1],
                )
            nc.vector.tensor_add(zb[:], zb[:], bg2_bot[:])
            psZ = ps_row.tile([1, RBOT], FP, tag="row")
            for t in range(RKB):
                nc.tensor.matmul(
                    psZ[0:1, t * P : (t + 1) * P],
                    zb[:, t : t + 1],
                    eye_t[:],
                    start=True,
                    stop=True,
                )
            nc.scalar.activation(s_row[0:1, RTOP:H], psZ[:], AF.Sigmoid, scale=-1.0)

            fin_a = sb.tile([1, RTOP], FP, tag="fin_a")
            nc.vector.tensor_mul(fin_a[:], outa_row[0:1, 0:RTOP], s_row[0:1, 0:RTOP])
            nc.sync.dma_start(fin.ap()[0:1, 0:RTOP], fin_a[:])
            fin_b = sb.tile([1, RBOT], FP, tag="fin_b")
            nc.vector.tensor_mul(fin_b[:], outa_row[0:1, RTOP:H], s_row[0:1, RTOP:H])
            nc.sync.dma_start(fin.ap()[0:1, RTOP:H], fin_b[:])

    nc.compile()
    return nc


def _make_in_maps_ref(inputs):
    Wexc = np.asarray(inputs["Wexc"], dtype=np.float32)
    Wg1 = np.asarray(inputs["Wg1"], dtype=np.float32)
    Wg2 = np.asarray(inputs["Wg2"], dtype=np.float32)
    blat = np.asarray(inputs["blat_e"], dtype=np.float32)
    bfb = np.asarray(inputs["bfb_e"], dtype=np.float32)
    bexc = np.asarray(inputs["bexc"], dtype=np.float32)
    bg1 = np.asarray(inputs["bg1"], dtype=np.float32)
    bg2 = np.asarray(inputs["bg2"], dtype=np.float32)
    tau = np.asarray(inputs["tau_exc"], dtype=np.float32)
    thr = np.asarray(inputs["threshold"], dtype=np.float32)

    s_idx = np.arange(H)
    rho = (s_idx % KT) * P + s_idx // KT
    eye = np.eye(P, dtype=np.float32)

    in_maps = []
    for c in range(NCORES):
        sl = slice(c * H, (c + 1) * H)
        srow = np.zeros((H,), np.float32)
        srow[0], srow[1] = tau[c], thr[c]
        bg2p = bg2[sl][rho]
        bg2_bot = bg2p[RTOP:].reshape(RKB, P).T.reshape(-1)
        row4 = np.concatenate([bg2p[:RTOP], bg2_bot])
        vecs = np.stack([blat[c, 0], bfb[c, 0], bexc[c, 0][rho], bg1[rho], row4, srow])
        w3pp = Wg2[sl][np.ix_(rho, rho)]
        in_maps.append(
            {
                "w1": np.ascontiguousarray(Wexc[c, 0]),
                "w2": np.ascontiguousarray(Wg1[:, sl][:, rho]),
                "w3t": np.ascontiguousarray(w3pp[0:RTOP, :].T),
                "w3n": np.ascontiguousarray(w3pp[RTOP:, :][:, rho]),
                "vecs": np.ascontiguousarray(vecs),
                "eye": eye,
            }
        )
    return in_maps


def _x0_is_zero(inputs):
    blat = np.asarray(inputs["blat_e"], dtype=np.float32)
    bfb = np.asarray(inputs["bfb_e"], dtype=np.float32)
    tau = np.asarray(inputs["tau_exc"], dtype=np.float32)
    thr = np.asarray(inputs["threshold"], dtype=np.float32)
    a = np.exp(-1.0 / tau)
    pre = (1.0 - a)[:, None] * (blat[:, 0] + bfb[:, 0]) - thr[:, None]
    return bool((pre <= 0.0).all())


def kernel(**inputs):
    if _x0_is_zero(inputs):
        if "nc_fast" not in _CACHE:
            _CACHE["nc_fast"] = _build_nc_fast()
        nc = _CACHE["nc_fast"]
        in_maps = _make_in_maps_fast(inputs)
        res = run_bass_kernel_spmd(nc, in_maps, core_ids=list(range(NCORES)))
        _CACHE["last_result"] = res
        chunks = [
            res.results[c]["final"].reshape(-1).astype(np.float32)
            for c in range(NCORES)
        ]
        return np.concatenate(chunks)

    if "nc_ref" not in _CACHE:
        _CACHE["nc_ref"] = _build_nc_ref()
    nc = _CACHE["nc_ref"]
    in_maps = _make_in_maps_ref(inputs)
    res = run_bass_kernel_spmd(nc, in_maps, core_ids=list(range(NCORES)))
    _CACHE["last_result"] = res
    chunks = []
    for c in range(NCORES):
        st = res.results[c]["final"].reshape(P, KT)
        chunks.append(np.ascontiguousarray(st.T).reshape(-1))
    return np.concatenate(chunks).astype(np.float32)


# revision 23
# speedup vs baseline: 1.7914x; 1.3337x over previous
"""Trainium2 Bass kernel for nn_CanonicalMicrocircuit (gnn_message_passing).

Math note: the reference module starts from all-zero recurrent state and only
returns `all_out * (1 - g)`, so every einsum against the zero state vanishes,
the inhibitory population and the inter-column lateral tensor are dead code,
and only layer 0 of the excitatory update survives:

    x0_c  = relu((1-exp(-1/tau_c)) * (blat_e[c,0] + bfb_e[c,0]) - thr_c)
    x0_c /= (||x0_c|| + 1e-8)
    out_c = relu(Wexc[c,0] @ x0_c + bexc[c,0])            # [H] per column
    h     = sum_c Wg1[:, cH:(c+1)H] @ out_c + bg1         # [H]
    r     = relu(h)
    g_c   = sigmoid(Wg2[cH:(c+1)H, :] @ r + bg2[cH:(c+1)H])
    final_c = out_c * (1 - g_c)                           # concat -> [C*H]

Fast path (v3): for the reference input distribution the layer-0 drive
(1-a)*(blat+bfb) tops out at ~0.26 while the threshold is 0.5, so x0 is
EXACTLY zero and out_c = relu(bexc[c,0]) on every column — verified on the
host for the actual inputs before this path is taken (guard below; the
collective-based general kernel is kept as fallback).  With out_all local to
every core there is no cross-core dataflow at all: each core redundantly
computes r = relu(bg1 + Wg1 @ out_all) (full Wg1 in fp8e4m3, 8MB, streamed
through the PE as 128 fp8 matmuls) and then its own output block
g_c/final_c (Wg2 block in bf16, split PE/DVE).  No collectives, no remote
DMA, no inter-core waits: immune to the multi-ms core-launch skew this
runtime exhibits for collective-free NEFFs, and each core's profile is just
its own ~35us of local work.  fp8 Wg1 + bf16 Wg2 gives rel err ~1.0e-2
against the fp32 reference (budget 2e-2); everything else stays fp32.

Fallback path: the previous collective-based kernel (ncfw AllGather of the
hp partials), used only if the host guard ever finds x0 != 0.
"""

import numpy as np
import ml_dtypes

import concourse.bass as bass
import concourse.bacc as bacc
import concourse.mybir as mybir
import concourse.tile as tile
from concourse.bass_utils import run_bass_kernel_spmd

C = 8
F = 512
L = 4
H = 1024
NCORES = 8
P = 128
KT = H // P       # 8 tiles per 1024 dim
KG = (C * H) // P  # 64 k-tiles over the 8192 contraction dim
NCH = 8            # wg1 DMA chunks (8 k-tiles each)
TOP = 512          # stage-C rows on the PE
BOT = H - TOP
KB = BOT // P
FP = mybir.dt.float32
BF = mybir.dt.bfloat16
F8 = mybir.dt.float8e4
NPBF = np.dtype(ml_dtypes.bfloat16)
NPF8 = np.dtype(ml_dtypes.float8_e4m3)

_CACHE = {}


DOUBLE_ROW = True


def _build_nc_fast():
    nc = bacc.Bacc(
        "TRN2",
        target_bir_lowering=False,
        debug=False,
        enable_asserts=False,
        num_devices=NCORES,
    )

    # All big weights arrive in host-prearranged per-partition-contiguous
    # layouts so every DMA is 128 flat descriptors (the naive "(k p) i"
    # rearranged loads spent 10us+ of ACT-queue time on descriptor gen).
    wg1p = nc.dram_tensor("wg1p", [P, KG * H], F8, kind="ExternalInput")
    wg2p = nc.dram_tensor("wg2p", [P, KT * H], BF, kind="ExternalInput")
    bxc = nc.dram_tensor("bxc", [P, KG], FP, kind="ExternalInput")
    vecs = nc.dram_tensor("vecs", [2, H], FP, kind="ExternalInput")
    # rows: 0 = bexc[c,0] (this block's out pre-relu), 1 = bg1
    b16 = nc.dram_tensor("b16", [1, H], BF, kind="ExternalInput")  # bg2[blk]
    fin = nc.dram_tensor("final", [1, H], FP, kind="ExternalOutput")

    AF = mybir.ActivationFunctionType
    ALU = mybir.AluOpType
    CHW = (KG * H) // NCH  # chunk width per partition (elements)

    with tile.TileContext(nc) as tc:
        with (
            tc.tile_pool(name="sb", bufs=1) as sb,
            tc.tile_pool(name="ps_row", bufs=1, space="PSUM") as ps_row,
            tc.tile_pool(name="ps_r", bufs=1, space="PSUM") as ps_r,
            tc.tile_pool(name="ps_c", bufs=1, space="PSUM") as ps_c,
        ):
            # ---- small loads first on the ACT ring ----
            vt = sb.tile([1, 2 * H], FP, tag="vecs")
            nc.scalar.dma_start(
                vt[:], vecs.ap().rearrange("a b -> (a b)").rearrange("(x n) -> x n", x=1)
            )
            bxc_t = sb.tile([P, KG], FP, tag="bxc")
            nc.scalar.dma_start(bxc_t[:], bxc.ap())
            b16_t = sb.tile([1, H], BF, tag="b16")
            nc.scalar.dma_start(b16_t[:], b16.ap())

            # act-table prewarm (Sigmoid/Relu/Copy) so no load lands mid-tail
            warm = sb.tile([1, 1], FP, tag="warm")
            nc.vector.memset(warm[:], 0.3)
            wj = sb.tile([1, 3], FP, tag="wj")
            for i, fn in enumerate((AF.Sigmoid, AF.Relu, AF.Copy)):
                nc.scalar.activation(wj[0:1, i : i + 1], warm[:], fn)

            one_11b = sb.tile([1, 1], BF, tag="one_11b")
            nc.vector.memset(one_11b[:], 1.0)

            # ---- out_all = relu(bexc[:,0,:]) in fp8 col form; out_c row ----
            # (emitted before the big DMA issues so the ACT queue converts
            # out8 at ~10us and the PE can start with the first chunk)
            oa_f = sb.tile([P, KG], FP, tag="oa_f")
            nc.vector.tensor_scalar_max(oa_f[:], bxc_t[:], 0.0)
            out8 = sb.tile([P, KG], F8, tag="out8")
            nc.scalar.activation(out8[:], oa_f[:], AF.Copy)
            out_row = sb.tile([1, H], FP, tag="out_row")
            nc.vector.tensor_scalar_max(out_row[:], vt[0:1, 0:H], 0.0)

            # ---- big loads: wg1 chunks alternate rings, wg2 halves last ----
            wch = []
            for a in range(NCH):
                t = sb.tile([P, CHW], F8, tag=f"wg1_{a}")
                eng = nc.sync if a % 2 == 0 else nc.scalar
                eng.dma_start(t[:], wg1p.ap()[:, a * CHW : (a + 1) * CHW])
                wch.append(t)
            w2t = sb.tile([P, KT * H], BF, tag="wg2")
            nc.sync.dma_start(w2t[:, 0 : KT * H // 2], wg2p.ap()[:, 0 : KT * H // 2])
            nc.scalar.dma_start(w2t[:, KT * H // 2 :], wg2p.ap()[:, KT * H // 2 :])

            # ---- r = relu(bg1 + Wg1 @ out_all) on the PE ----
            NQ = 4  # 256-wide output quarters
            psq = []
            for qq in range(NQ):
                tq = ps_r.tile([1, TOP], FP, tag=f"rq{qq}")
                psq.append(tq)
            if DOUBLE_ROW:
                # wg1p chunk layout: [p, (a q s j)]; one DoubleRow mm per
                # (pair a, quarter q) contracts k-blocks 2a and 2a+1.  The
                # stationary pair comes from out8's even/odd k-planes
                # ([p, s, a] with 32B plane stride — the ISA wants dim1
                # Num=2 with a 16B-aligned stride on both operands).
                out8v = out8[:].rearrange("p (s a) -> p s a", s=2)
                for a in range(KG // 2):
                    ch, b = wch[a // 4], a % 4
                    for qq in range(NQ):
                        off = b * (NQ * 512) + qq * 512
                        nc.tensor.matmul(
                            psq[qq][0:1, 0:256],
                            out8v[:, :, a : a + 1],
                            ch[:, off : off + 512].rearrange(
                                "p (s j) -> p s j", s=2
                            ),
                            start=(a == 0),
                            stop=(a == KG // 2 - 1),
                            perf_mode=mybir.MatmulPerfMode.DoubleRow,
                        )
            else:
                for a in range(KG // 2):
                    ch, b = wch[a // 4], a % 4
                    for qq in range(NQ):
                        off = b * (NQ * 512) + qq * 512
                        for s in range(2):
                            col = s * (KG // 2) + a  # out8 is [p, (s a)]
                            nc.tensor.matmul(
                                psq[qq][0:1, 0:256],
                                out8[:, col : col + 1],
                                ch[:, off + s * 256 : off + (s + 1) * 256],
                                start=(a == 0 and s == 0),
                                stop=(a == KG // 2 - 1 and s == 1),
                            )

            # r (+bg1, relu) -> bf16 row; quarters q are j-ranges [256q, 256q+256)
            r_row = sb.tile([1, H], FP, tag="r_row")
            for qq in range(NQ):
                sl = slice(qq * 256, (qq + 1) * 256)
                nc.vector.tensor_add(
                    r_row[0:1, sl], psq[qq][0:1, 0:256], vt[0:1, H + qq * 256 : H + (qq + 1) * 256]
                )
            rrowb = sb.tile([1, H], BF, tag="rrowb")
            nc.scalar.activation(rrowb[:], r_row[:], AF.Relu)

            # ---- r row -> col (PE ones-transpose) ----
            psc = ps_row.tile([P, KT], FP, tag="rcol")
            for t in range(KT):
                nc.tensor.matmul(
                    psc[:, t : t + 1],
                    rrowb[0:1, t * P : (t + 1) * P],
                    one_11b[:],
                    start=True,
                    stop=True,
                )
            r_colb = sb.tile([P, KT], BF, tag="r_colb")
            nc.scalar.activation(r_colb[:], psc[:], AF.Copy)

            # ---- stage C fully on PE: s = sigmoid(-(W2blk @ r + bg2)) ----
            # half h=0 completes first so its sigmoid/mul/store overlap h=1.
            s_row = sb.tile([1, H], FP, tag="s_row")
            fin_h = []
            for h in range(2):
                psC = ps_c.tile([1, TOP], FP, tag=f"c{h}")
                for k in range(KT):
                    nc.tensor.matmul(
                        psC[:],
                        r_colb[:, k : k + 1],
                        w2t[:, k * H + h * TOP : k * H + (h + 1) * TOP],
                        start=(k == 0),
                        stop=False,
                    )
                nc.tensor.matmul(
                    psC[:], one_11b[:], b16_t[0:1, h * TOP : (h + 1) * TOP],
                    start=False, stop=True,
                )
                sl = slice(h * TOP, (h + 1) * TOP)
                nc.scalar.activation(s_row[0:1, sl], psC[:], AF.Sigmoid, scale=-1.0)
                ft = sb.tile([1, TOP], FP, tag=f"fin{h}")
                nc.vector.tensor_mul(ft[:], out_row[0:1, sl], s_row[0:1, sl])
                nc.sync.dma_start(fin.ap()[0:1, sl], ft[:])
                fin_h.append(ft)

    nc.compile()
    return nc


def _make_in_maps_fast(inputs):
    bexc = np.asarray(inputs["bexc"], dtype=np.float32)
    bg1 = np.asarray(inputs["bg1"], dtype=np.float32)
    bg2 = np.asarray(inputs["bg2"], dtype=np.float32)
    Wg1 = np.asarray(inputs["Wg1"], dtype=np.float32)
    Wg2 = np.asarray(inputs["Wg2"], dtype=np.float32)

    # [p, (a q s j)] : pair a, out-quarter q, k-parity s, j within quarter
    T = np.ascontiguousarray(Wg1.T).astype(NPF8)
    wg1p = np.ascontiguousarray(
        T.reshape(32, 2, P, 4, 256).transpose(2, 0, 3, 1, 4).reshape(P, -1)
    )
    # bxc[p, s*32+a] = out_flat[(2a+s)*128+p]  (even/odd k-planes for DoubleRow)
    bx_flat = bexc[:, 0, :].reshape(-1)
    bxc = np.ascontiguousarray(
        bx_flat.reshape(KG // 2, 2, P).transpose(2, 1, 0).reshape(P, KG)
    )

    in_maps = []
    for c in range(NCORES):
        sl = slice(c * H, (c + 1) * H)
        w2 = np.ascontiguousarray(Wg2[sl].T).astype(NPBF)  # [H(k), H(i)]
        wg2p = np.ascontiguousarray(
            w2.reshape(KT, P, H).transpose(1, 0, 2).reshape(P, -1)
        )
        vecs = np.stack([bexc[c, 0], bg1])
        in_maps.append(
            {
                "wg1p": wg1p,
                "wg2p": wg2p,
                "bxc": bxc,
                "vecs": np.ascontiguousarray(vecs),
                "b16": np.ascontiguousarray(bg2[sl])[None, :].astype(NPBF),
            }
        )
    return in_maps


# ---------------------------------------------------------------------------
# Fallback: collective-based general kernel (previous proven version), used
# only if the x0==0 guard fails.
# ---------------------------------------------------------------------------

HI = 256
RTOP = 384  # stage-C rows on the PE (fallback layout)
RBOT = H - RTOP
RKB = RBOT // P


def _build_nc_ref():
    nc = bacc.Bacc(
        "TRN2",
        target_bir_lowering=False,
        debug=False,
        enable_asserts=False,
        num_devices=NCORES,
    )

    w1 = nc.dram_tensor("w1", [H, H], FP, kind="ExternalInput")
    w2 = nc.dram_tensor("w2", [H, H], FP, kind="ExternalInput")
    w3t = nc.dram_tensor("w3t", [H, RTOP], FP, kind="ExternalInput")
    w3n = nc.dram_tensor("w3n", [RBOT, H], FP, kind="ExternalInput")
    vecs = nc.dram_tensor("vecs", [6, H], FP, kind="ExternalInput")
    eye = nc.dram_tensor("eye", [P, P], FP, kind="ExternalInput")
    fin = nc.dram_tensor("final", [1, H], FP, kind="ExternalOutput")

    AF = mybir.ActivationFunctionType
    ALU = mybir.AluOpType

    with tile.TileContext(nc) as tc:
        with (
            tc.tile_pool(name="sb", bufs=1) as sb,
            tc.tile_pool(name="jk", bufs=2) as jk,
            tc.tile_pool(name="ps_row", bufs=3, space="PSUM") as ps_row,
            tc.tile_pool(name="ps_tp", bufs=1, space="PSUM") as ps_tp,
            tc.tile_pool(name="dram", bufs=1, space="DRAM") as dram,
        ):
            def load_nat_pairs(name, dram_t):
                tiles = []
                for a in range(KT // 4):
                    t = sb.tile([P, 4, H], FP, tag=f"{name}{a}")
                    src = dram_t.ap()[4 * a * P : 4 * (a + 1) * P, :].rearrange(
                        "(t p) i -> p t i", p=P
                    )
                    nc.sync.dma_start(t[:], src)
                    tiles.append(t)
                return tiles

            w1_t = load_nat_pairs("w1", w1)
            w2_t = load_nat_pairs("w2", w2)
            w3t_t = sb.tile([P, KT, RTOP], FP, tag="w3t")
            nc.sync.dma_start(w3t_t[:], w3t.ap().rearrange("(k p) i -> p k i", p=P))
            w3n_t = sb.tile([P, RKB, H], FP, tag="w3n")
            nc.sync.dma_start(w3n_t[:], w3n.ap().rearrange("(t p) i -> p t i", p=P))

            vt = sb.tile([1, 6 * H], FP, tag="vecs")
            nc.scalar.dma_start(
                vt[:], vecs.ap().rearrange("a b -> (a b)").rearrange("(x n) -> x n", x=1)
            )
            bexc_col = sb.tile([P, KT], FP, tag="bexc_col")
            nc.scalar.dma_start(
                bexc_col[:], vecs.ap()[2].rearrange("(p t) -> p t", p=P)
            )
            bg2_bot = sb.tile([P, RKB], FP, tag="bg2_bot")
            nc.scalar.dma_start(
                bg2_bot[:], vecs.ap()[4][RTOP:H].rearrange("(p t) -> p t", p=P)
            )

            eye_t = sb.tile([P, P], FP, tag="eye")
            nc.scalar.dma_start(eye_t[:], eye.ap())
            ones_8 = sb.tile([KT, 1], FP, tag="ones_8")
            one_11 = sb.tile([1, 1], FP, tag="one_11")
            nc.vector.memset(ones_8[:], 1.0)
            nc.vector.memset(one_11[:], 1.0)

            rt = sb.tile([1, 1], FP, tag="rt")
            nc.vector.reciprocal(rt[:], vt[0:1, 5 * H : 5 * H + 1])
            ea = sb.tile([1, 1], FP, tag="ea")
            nc.scalar.activation(ea[:], rt[:], AF.Exp, scale=-1.0)
            oma = sb.tile([1, 1], FP, tag="oma")
            nc.scalar.activation(oma[:], ea[:], AF.Copy, scale=-1.0, bias=1.0)
            nthr = sb.tile([1, 1], FP, tag="nthr")
            nc.scalar.activation(nthr[:], vt[0:1, 5 * H + 1 : 5 * H + 2], AF.Copy, scale=-1.0)

            xr = sb.tile([1, H], FP, tag="xr")
            nc.vector.tensor_add(xr[:], vt[0:1, 0:H], vt[0:1, H : 2 * H])
            nc.vector.tensor_scalar(
                xr[:], xr[:], oma[:], nthr[:], op0=ALU.mult, op1=ALU.add
            )
            nc.vector.tensor_scalar_max(xr[:], xr[:], 0.0)
            ssq = sb.tile([1, 1], FP, tag="ssq")
            sqj = jk.tile([1, H], FP, tag="sqj")
            nc.vector.scalar_tensor_tensor(
                sqj[:], xr[:], 1.0, xr[:], op0=ALU.mult, op1=ALU.mult,
                accum_out=ssq[:],
            )
            nrm = sb.tile([1, 1], FP, tag="nrm")
            nc.scalar.activation(nrm[:], ssq[:], AF.Sqrt)
            nc.scalar.activation(nrm[:], nrm[:], AF.Copy, bias=1e-8)
            inv = sb.tile([1, 1], FP, tag="inv")
            nc.vector.reciprocal(inv[:], nrm[:])
            nc.vector.tensor_scalar_mul(xr[:], xr[:], inv[:])

            xb = sb.tile([P, H], FP, tag="xb")
            nc.gpsimd.partition_broadcast(xb[:], xr[0:1, :])

            def matvec_nat(tiles, vb, acc):
                for t in range(KT):
                    w_ap = tiles[t // 4][:, t % 4, :]
                    junk = jk.tile([P, H], FP, tag="jv")
                    nc.vector.scalar_tensor_tensor(
                        junk[:], w_ap, 1.0, vb[:], op0=ALU.mult, op1=ALU.mult,
                        accum_out=acc[:, t : t + 1],
                    )

            outa = sb.tile([P, KT], FP, tag="outa")
            matvec_nat(w1_t, xb, outa)
            nc.vector.tensor_add(outa[:], outa[:], bexc_col[:])
            nc.vector.tensor_scalar_max(outa[:], outa[:], 0.0)
            outa_row = sb.tile([1, H], FP, tag="outa_row")
            nc.scalar.dma_start(outa_row[:], outa[:])
            xb2 = sb.tile([P, H], FP, tag="xb2")
            nc.gpsimd.partition_broadcast(xb2[:], outa_row[0:1, :])

            bg1_col = sb.tile([P, KT], FP, tag="bg1_col")
            nc.scalar.dma_start(
                bg1_col[:], vecs.ap()[3].rearrange("(p t) -> p t", p=P)
            )
            hp = sb.tile([P, KT], FP, tag="hp")
            matvec_nat(w2_t, xb2, hp)
            nc.vector.scalar_tensor_tensor(
                hp[:], bg1_col[:], 0.125, hp[:], op0=ALU.mult, op1=ALU.add
            )

            cc_in = dram.tile([1, H], FP, tag="cc_in")
            cc_out = dram.tile([NCORES, H], FP, tag="cc_out")
            nc.scalar.dma_start(cc_in[:], hp[:])
            nc.gpsimd.collective_compute(
                "AllGather",
                ALU.bypass,
                replica_groups=[list(range(NCORES))],
                ins=[cc_in[:]],
                outs=[cc_out[:]],
            )
            agt_a = sb.tile([NCORES, H // 2], FP, tag="agt_a")
            nc.scalar.dma_start(agt_a[:], cc_out[:, 0 : H // 2])
            agt_b = sb.tile([NCORES, H // 2], FP, tag="agt_b")
            nc.sync.dma_start(agt_b[:], cc_out[:, H // 2 : H])
            psRc = ps_tp.tile([P, KT], FP, tag="tp")
            for t in range(KT):
                half = agt_a if t < 4 else agt_b
                nc.tensor.matmul(
                    psRc[:, t : t + 1],
                    half[:, (t % 4) * P : (t % 4 + 1) * P],
                    ones_8[:],
                    start=True,
                    stop=True,
                )
            r_col = sb.tile([P, KT], FP, tag="r_col")
            nc.scalar.activation(r_col[:], psRc[:], AF.Relu)

            s_row = sb.tile([1, H], FP, tag="s_row")
            ps = ps_row.tile([1, RTOP], FP, tag="row")
            for k in range(KT):
                nc.tensor.matmul(
                    ps[:],
                    r_col[:, k : k + 1],
                    w3t_t[:, k, :],
                    start=(k == 0),
                    stop=False,
                )
            nc.tensor.matmul(
                ps[:], one_11[:], vt[0:1, 4 * H : 4 * H + RTOP], start=False, stop=True
            )
            nc.scalar.activation(s_row[0:1, 0:RTOP], ps[:], AF.Sigmoid, scale=-1.0)

            rrow2 = sb.tile([1, H], FP, tag="rrow2")
            nc.scalar.dma_start(rrow2[:], r_col[:])
            xb3 = sb.tile([P, H], FP, tag="xb3")
            nc.gpsimd.partition_broadcast(xb3[:], rrow2[0:1, :])
            zb = sb.tile([P, RKB], FP, tag="zb")
            for t in range(RKB):
                junk = jk.tile([P, H], FP, tag="jv")
                nc.vector.scalar_tensor_tensor(
                    junk[:], w3n_t[:, t, :], 1.0, xb3[:], op0=ALU.mult, op1=ALU.mult,
                    accum_out=zb[:, t : t + 1],
                )
            nc.vector.tensor_add(zb[:], zb[:], bg2_bot[:])
            psZ = ps_row.tile([1, RBOT], FP, tag="row")
            for t in range(RKB):
                nc.tensor.matmul(
                    psZ[0:1, t * P : (t + 1) * P],
                    zb[:, t : t + 1],
                    eye_t[:],
                    start=True,
                    stop=True,
                )
            nc.scalar.activation(s_row[0:1, RTOP:H], psZ[:], AF.Sigmoid, scale=-1.0)

            fin_a = sb.tile([1, RTOP], FP, tag="fin_a")
            nc.vector.tensor_mul(fin_a[:], outa_row[0:1, 0:RTOP], s_row[0:1, 0:RTOP])
            nc.sync.dma_start(fin.ap()[0:1, 0:RTOP], fin_a[:])
            fin_b = sb.tile([1, RBOT], FP, tag="fin_b")
            nc.vector.tensor_mul(fin_b[:], outa_row[0:1, RTOP:H], s_row[0:1, RTOP:H])
            nc.sync.dma_start(fin.ap()[0:1, RTOP:H], fin_b[:])

    nc.compile()
    return nc


def _make_in_maps_ref(inputs):
    Wexc = np.asarray(inputs["Wexc"], dtype=np.float32)
    Wg1 = np.asarray(inputs["Wg1"], dtype=np.float32)
    Wg2 = np.asarray(inputs["Wg2"], dtype=np.float32)
    blat = np.asarray(inputs["blat_e"], dtype=np.float32)
    bfb = np.asarray(inputs["bfb_e"], dtype=np.float32)
    bexc = np.asarray(inputs["bexc"], dtype=np.float32)
    bg1 = np.asarray(inputs["bg1"], dtype=np.float32)
    bg2 = np.asarray(inputs["bg2"], dtype=np.float32)
    tau = np.asarray(inputs["tau_exc"], dtype=np.float32)
    thr = np.asarray(inputs["threshold"], dtype=np.float32)

    s_idx = np.arange(H)
    rho = (s_idx % KT) * P + s_idx // KT
    eye = np.eye(P, dtype=np.float32)

    in_maps = []
    for c in range(NCORES):
        sl = slice(c * H, (c + 1) * H)
        srow = np.zeros((H,), np.float32)
        srow[0], srow[1] = tau[c], thr[c]
        bg2p = bg2[sl][rho]
        bg2_bot = bg2p[RTOP:].reshape(RKB, P).T.reshape(-1)
        row4 = np.concatenate([bg2p[:RTOP], bg2_bot])
        vecs = np.stack([blat[c, 0], bfb[c, 0], bexc[c, 0][rho], bg1[rho], row4, srow])
        w3pp = Wg2[sl][np.ix_(rho, rho)]
        in_maps.append(
            {
                "w1": np.ascontiguousarray(Wexc[c, 0]),
                "w2": np.ascontiguousarray(Wg1[:, sl][:, rho]),
                "w3t": np.ascontiguousarray(w3pp[0:RTOP, :].T),
                "w3n": np.ascontiguousarray(w3pp[RTOP:, :][:, rho]),
                "vecs": np.ascontiguousarray(vecs),
                "eye": eye,
            }
        )
    return in_maps


def _x0_is_zero(inputs):
    blat = np.asarray(inputs["blat_e"], dtype=np.float32)
    bfb = np.asarray(inputs["bfb_e"], dtype=np.float32)
    tau = np.asarray(inputs["tau_exc"], dtype=np.float32)
    thr = np.asarray(inputs["threshold"], dtype=np.float32)
    a = np.exp(-1.0 / tau)
    pre = (1.0 - a)[:, None] * (blat[:, 0] + bfb[:, 0]) - thr[:, None]
    return bool((pre <= 0.0).all())


def kernel(**inputs):
    if _x0_is_zero(inputs):
        if "nc_fast" not in _CACHE:
            _CACHE["nc_fast"] = _build_nc_fast()
        nc = _CACHE["nc_fast"]
        in_maps = _make_in_maps_fast(inputs)
        res = run_bass_kernel_spmd(nc, in_maps, core_ids=list(range(NCORES)))
        _CACHE["last_result"] = res
        chunks = [
            res.results[c]["final"].reshape(-1).astype(np.float32)
            for c in range(NCORES)
        ]
        return np.concatenate(chunks)

    if "nc_ref" not in _CACHE:
        _CACHE["nc_ref"] = _build_nc_ref()
    nc = _CACHE["nc_ref"]
    in_maps = _make_in_maps_ref(inputs)
    res = run_bass_kernel_spmd(nc, in_maps, core_ids=list(range(NCORES)))
    _CACHE["last_result"] = res
    chunks = []
    for c in range(NCORES):
        st = res.results[c]["final"].reshape(P, KT)
        chunks.append(np.ascontiguousarray(st.T).reshape(-1))
    return np.concatenate(chunks).astype(np.float32)
